# revision 17
# baseline (speedup 1.0000x reference)
"""Trainium2 Bass kernel for nn_Dihedral (gnn_message_passing, 8 NeuronCores).

kernel(**inputs) -> [256] f32 per-batch dihedral energies.

Design v3 — gather-free streaming, engine-split compute. mapping columns are
consecutive-atom windows (b..b+3), so every per-dihedral quantity except the
batch label is a function of the window start. The host builds, per core, a
batch-sorted per-dihedral stream of 15 bf16 field planes
    [dr1 (3), dr2 (3), dr3 (3), A1..A3, B1'..B3']
(A = -k*cos(th0), B' = sign/2x-folded -k*sin(th0) — the same type-table fold
as v1; the angle-independent C = sum_k term is summed host-side) laid out
tile-contiguously in DRAM, so the device does pure SEQUENTIAL DMA (no
dma_gather, 30B/dihedral vs 256B in v1). Bins are padded to 32-element
blocks along partitions.

Device per core, per [128, F] tile: one dma_start pulls all 15 planes; the
columns are SPLIT between DVE (bf16 2x_1p) and gpsimd, each running the full
torsion chain independently on its slice (no cross-engine dependency
stalls); ACT does squares/sqrts/copies for both slices; PE reduces
32-partition blocks with a [128,4] indicator matmul. The torsion uses the
xy-polynomial form (cos/sin of d*phi expanded in x = n1.n2,
Y = (dr1.n2)|dr2|, u = 1/r — no trig tables). Host: bincount block sums
into 256 bins, sum the 8 per-core partials (the all-reduce of the
sum-sharded output).
"""

import os
import sys
import numpy as np

if "/opt/trn_rl_repo" not in sys.path:
    sys.path.insert(0, "/opt/trn_rl_repo")

import concourse.bass as bass
import concourse.bacc as bacc
import concourse.mybir as mybir
import concourse.tile as tile
from concourse.library_config import standard as std_lib
from concourse.tile_rust import add_dep_helper
import ml_dtypes

P = 128
NCORES = 8
QUANT = 32           # bin padding quantum == PE group size
NGRP = P // QUANT    # 4 partial sums per column
NPLANES = 15
NB = 256


# --------------------------------------------------------------------------
# host-side prep
# --------------------------------------------------------------------------

def build_window_fields(pos, atom_types, thetas, ks):
    """([15, NW] f32 per-window field planes, [NW] f32 C values)."""
    NW = pos.shape[0] - 3
    t3 = thetas.reshape(3, -1).astype(np.float64)
    k3 = ks.reshape(3, -1).astype(np.float64)
    A = (-(k3 * np.cos(t3))).astype(np.float32)      # [3, 390625]
    B = (-(k3 * np.sin(t3))).astype(np.float32)
    C = k3.sum(axis=0).astype(np.float32)
    ty = np.asarray(atom_types).astype(np.int64)
    T4 = ((ty[:NW] * 25 + ty[1:NW + 1]) * 25 + ty[2:NW + 2]) * 25 + ty[3:NW + 3]
    f = np.empty((NPLANES, NW), dtype=np.float32)
    f[0:3] = (pos[1:NW + 1] - pos[0:NW]).T
    f[3:6] = (pos[2:NW + 2] - pos[1:NW + 1]).T
    f[6:9] = (pos[3:NW + 3] - pos[2:NW + 2]).T
    f[9] = A[0, T4]; f[10] = A[1, T4]; f[11] = A[2, T4]
    f[12] = -B[0, T4]; f[13] = -2.0 * B[1, T4]; f[14] = -B[2, T4]
    return f, C[T4]


def plan_streams(base, batch, n_win, F):
    """Per-core batch-sorted window-index streams, bins padded to QUANT,
    common NTILES across cores."""
    SUBW = (n_win + NCORES - 1) // NCORES
    core_of = base // SUBW

    idx_streams = []
    lab_streams = []
    for c in range(NCORES):
        sel = np.nonzero(core_of == c)[0]        # batch-sorted already
        lab = batch[sel]
        cnt = np.bincount(lab, minlength=NB)
        pcnt = ((cnt + QUANT - 1) // QUANT) * QUANT
        total = int(pcnt.sum())
        idx_out = np.full(total, -1, dtype=np.int64)
        ends = np.cumsum(pcnt)
        starts = ends - pcnt
        within = np.arange(len(sel)) - np.repeat(np.cumsum(cnt) - cnt, cnt)
        idx_out[starts[lab] + within] = base[sel]
        blk_lab = np.repeat(np.arange(NB, dtype=np.int32), pcnt // QUANT)
        idx_streams.append(idx_out)
        lab_streams.append(blk_lab)

    max_cols = max((len(s) + P - 1) // P for s in idx_streams)
    NTILES = (max_cols + F - 1) // F
    NCOLS = NTILES * F

    streams = np.full((NCORES, NCOLS * P), -1, dtype=np.int64)
    blk_labels = np.full((NCORES, NCOLS * NGRP), -1, dtype=np.int32)
    for c in range(NCORES):
        streams[c, :len(idx_streams[c])] = idx_streams[c]
        blk_labels[c, :len(lab_streams[c])] = lab_streams[c]
    return streams, blk_labels, NTILES


def build_core_tables(fields, streams, NTILES, F, np_dtype):
    """[NCORES, NTILES*P, NPLANES*F] plane tables (tile-contiguous DRAM)."""
    tables = np.empty((NCORES, NTILES * P, NPLANES * F), dtype=np_dtype)
    for c in range(NCORES):
        w = streams[c]
        dummy = w < 0
        vals = fields[:, np.where(dummy, 0, w)]          # [15, NCOLS*P]
        if dummy.any():
            vals[9:15][:, dummy] = 0.0                   # V = 0 for padding
        # stream s = t*(P*F) + col*P + p  ->  dram[t*P + p, k*F + col]
        v4 = vals.reshape(NPLANES, NTILES, F, P)
        tables[c] = (v4.transpose(1, 3, 0, 2)
                     .reshape(NTILES * P, NPLANES * F).astype(np_dtype))
    return tables


# --------------------------------------------------------------------------
# device program
# --------------------------------------------------------------------------

def build_program(NTILES, F, repeat=1, dtype="bf16", split=None,
                  gat_bufs=None, tmp_bufs=None):
    if split is None:
        split = float(os.environ.get("DK_SPLIT", "1.0"))
    if gat_bufs is None:
        gat_bufs = int(os.environ.get("DK_GATB", "2"))
    if tmp_bufs is None:
        tmp_bufs = int(os.environ.get("DK_TMPB", "1"))
    f32 = mybir.dt.float32
    dt = mybir.dt.bfloat16 if dtype == "bf16" else f32
    Alu = mybir.AluOpType
    Act = mybir.ActivationFunctionType
    NCOLS = NTILES * F
    S = (int(F * split) // 4) * 4            # DVE columns; rest go to Pool

    nc = bacc.Bacc("TRN2", target_bir_lowering=False, debug=False)
    tbl = nc.dram_tensor("tbl", [NTILES * P, NPLANES * F], dt,
                         kind="ExternalInput").ap()
    out = nc.dram_tensor("out", [NGRP, NCOLS], f32, kind="ExternalOutput").ap()

    with tile.TileContext(nc) as tc:
        with (
            tc.tile_pool(name="gat", bufs=gat_bufs) as gat_pool,
            tc.tile_pool(name="tmp", bufs=tmp_bufs) as tmp_pool,
            tc.tile_pool(name="cst", bufs=1) as cst_pool,
            tc.tile_pool(name="ps", bufs=2, space="PSUM") as ps_pool,
        ):
            lib_inst = nc.gpsimd.load_library(std_lib)

            grp = cst_pool.tile([P, NGRP], dt)
            nc.gpsimd.memset(grp[:], 0.0)
            for g in range(NGRP):
                nc.gpsimd.memset(grp[g * QUANT:(g + 1) * QUANT, g:g + 1], 1.0)

            bs = cst_pool.tile([NGRP, NCOLS], f32)

            first_pool_op = [None]

            def do_side(g, side, lo, n):
                """Full torsion chain for columns [lo, lo+n) of tile g;
                tensor ops on DVE, squares/sqrt/affine on ACT."""
                def fld(k):
                    return g[:, k * F + lo:k * F + lo + n]

                def T(tag, d=dt):
                    return tmp_pool.tile([P, n], d, tag=f"{tag}_{side}",
                                         name=f"{tag}_{side}")

                def tt(o, i0, i1, op):
                    nc.vector.tensor_tensor(out=o, in0=i0, in1=i1, op=op)

                ax, ay, az = fld(0), fld(1), fld(2)
                bx, by, bz = fld(3), fld(4), fld(5)
                cx, cy, cz = fld(6), fld(7), fld(8)

                cs = T("cs")
                def cross1(o, m1, m2, m3, m4):
                    tt(o[:], m1, m2, Alu.mult)
                    tt(cs[:], m3, m4, Alu.mult)
                    tt(o[:], o[:], cs[:], Alu.subtract)

                n1x = T("n1x"); cross1(n1x, ay, bz, az, by)
                n1y = T("n1y"); cross1(n1y, az, bx, ax, bz)
                n1z = T("n1z"); cross1(n1z, ax, by, ay, bx)
                n2x = T("n2x"); cross1(n2x, by, cz, bz, cy)
                n2y = T("n2y"); cross1(n2y, bz, cx, bx, cz)
                n2z = T("n2z"); cross1(n2z, bx, cy, by, cx)

                def dot3(o, u0, u1, u2_, v0, v1, v2):
                    st = T("dt")
                    tt(o[:], u0, v0, Alu.mult)
                    tt(st[:], u1, v1, Alu.mult)
                    tt(o[:], o[:], st[:], Alu.add)
                    tt(st[:], u2_, v2, Alu.mult)
                    tt(o[:], o[:], st[:], Alu.add)

                x = T("x"); dot3(x, n1x[:], n1y[:], n1z[:], n2x[:], n2y[:], n2z[:])
                D = T("D"); dot3(D, ax, ay, az, n2x[:], n2y[:], n2z[:])

                w = T("w"); wt = T("wt")
                nc.scalar.square(w[:], bx)
                nc.scalar.square(wt[:], by)
                tt(w[:], w[:], wt[:], Alu.add)
                nc.scalar.square(wt[:], bz)
                tt(w[:], w[:], wt[:], Alu.add)
                L = T("L"); nc.scalar.sqrt(L[:], w[:])
                Y = T("Y"); tt(Y[:], D[:], L[:], Alu.mult)

                xx = T("xx"); nc.scalar.square(xx[:], x[:])
                YY = T("YY"); nc.scalar.square(YY[:], Y[:])
                q = T("q"); tt(q[:], xx[:], YY[:], Alu.add)
                r = T("r", f32); nc.scalar.sqrt(r[:], q[:])
                uf = T("uf", f32)
                nc.vector.reciprocal_approx_fast(uf[:], r[:])
                u = T("u"); nc.scalar.copy(u[:], uf[:])

                # normalized X = cos(phi), Yh = -sin(phi); X^2 + Yh^2 = 1
                Act_ = mybir.ActivationFunctionType
                X = T("X"); tt(X[:], x[:], u[:], Alu.mult)
                Yh = T("Yh"); tt(Yh[:], Y[:], u[:], Alu.mult)
                YY1 = T("YY1"); nc.scalar.square(YY1[:], Yh[:])
                c2 = T("c2")
                nc.scalar.activation(c2[:], YY1[:], Act_.Copy, bias=1.0,
                                     scale=-2.0)
                c3a = T("c3a")
                nc.scalar.activation(c3a[:], YY1[:], Act_.Copy, bias=1.0,
                                     scale=-4.0)
                s3a = T("s3a")
                nc.scalar.activation(s3a[:], YY1[:], Act_.Copy, bias=3.0,
                                     scale=-4.0)
                s2 = T("s2"); tt(s2[:], X[:], Yh[:], Alu.mult)
                c3 = T("c3"); tt(c3[:], c3a[:], X[:], Alu.mult)
                s3 = T("s3"); tt(s3[:], s3a[:], Yh[:], Alu.mult)

                # six Fourier terms; their sum is absorbed into the PSUM
                # accumulation of the block-sum matmuls
                npool = int(os.environ.get("DK_NPOOL", "0"))
                terms = []
                for i, (coef, trig) in enumerate([
                        (9, X), (12, Yh), (10, c2), (13, s2),
                        (11, c3), (14, s3)]):
                    tm = tmp_pool.tile([P, n], dt, tag=f"tm{i}_{side}",
                                       name=f"tm{i}_{side}")
                    if i < npool:
                        pi = nc.gpsimd.tensor_tensor(
                            out=tm[:], in0=fld(coef), in1=trig[:], op=Alu.mult)
                        if first_pool_op[0] is None:
                            first_pool_op[0] = pi
                            add_dep_helper(lib_inst.ins, pi.ins, sync=False,
                                           reason="library before pool ops")
                    else:
                        tt(tm[:], fld(coef), trig[:], Alu.mult)
                    terms.append(tm)
                return terms

            def do_tile(t):
                g = gat_pool.tile([P, NPLANES * F], dt, tag="g")
                nc.sync.dma_start(out=g[:], in_=tbl[t * P:(t + 1) * P, :])
                terms = do_side(g[:], "d", 0, F)
                pt = ps_pool.tile([NGRP, F], f32, tag="ps", name="ps")
                for c0 in range(0, F, 512):
                    c1 = min(c0 + 512, F)
                    for i, tm in enumerate(terms):
                        nc.tensor.matmul(out=pt[:, c0:c1], lhsT=grp[:],
                                         rhs=tm[:, c0:c1],
                                         start=(i == 0), stop=(i == 5))
                nc.scalar.activation(bs[:, t * F:(t + 1) * F], pt[:], Act.Copy)

            def body():
                for t in range(NTILES):
                    do_tile(t)

            if repeat > 1:
                with tc.For_i(0, repeat, 1):
                    body()
            else:
                body()

            nc.sync.dma_start(out=out[:], in_=bs[:])
    nc.compile()
    return nc


# --------------------------------------------------------------------------
# end to end
# --------------------------------------------------------------------------

def prepare(inputs, F=1024, dtype="bf16"):
    pos = np.asarray(inputs["pos"], dtype=np.float32)
    ty = np.asarray(inputs["atom_types"])
    mapping = np.asarray(inputs["mapping"])
    batch = np.asarray(inputs["mapping_batch"]).astype(np.int64)
    base = np.asarray(mapping[0]).astype(np.int64)
    assert all(np.array_equal(np.asarray(mapping[j]), base + j)
               for j in range(1, 4)), "mapping not consecutive; fast path invalid"
    n_win = pos.shape[0] - 3
    fields, Cw = build_window_fields(pos, ty, np.asarray(inputs["thetas"]),
                                     np.asarray(inputs["ks"]))
    streams, blk_labels, NTILES = plan_streams(base, batch, n_win, F)
    np_dtype = ml_dtypes.bfloat16 if dtype == "bf16" else np.float32
    tables = build_core_tables(fields, streams, NTILES, F, np_dtype)
    # angle-independent sum_k term, accumulated host-side
    energy_C = np.bincount(batch, weights=Cw[base].astype(np.float64),
                           minlength=NB)
    plan = dict(NTILES=NTILES, F=F, blk_labels=blk_labels, dtype=dtype,
                energy_C=energy_C)
    return plan, tables


def finish(plan, outs, n_batch=NB):
    """outs: list per core of [NGRP, NCOLS] block sums -> [256] energy."""
    energy = plan["energy_C"].copy()
    for c in range(NCORES):
        bsums = np.asarray(outs[c])          # [NGRP, NCOLS]
        lab = plan["blk_labels"][c]          # [NCOLS*NGRP], -1 = padding
        vals = bsums.T.ravel()               # block (col, grp) order
        m = lab >= 0
        energy += np.bincount(lab[m], weights=vals[m].astype(np.float64),
                              minlength=n_batch)
    return energy.astype(np.float32)


def _kernel_numpy_fallback(pos, atom_types, mapping, mapping_batch, thetas, ks):
    # Correctness safety net for non-consecutive mappings (never expected).
    p0, p1 = pos[mapping[0]], pos[mapping[1]]
    p2, p3 = pos[mapping[2]], pos[mapping[3]]
    dr1, dr2, dr3 = p1 - p0, p2 - p1, p3 - p2
    n1 = np.cross(dr1, dr2); n2 = np.cross(dr2, dr3)
    m1 = np.cross(n1, dr2 / np.linalg.norm(dr2, axis=-1, keepdims=True))
    x = np.sum(n1 * n2, -1); y = np.sum(m1 * n2, -1)
    theta = np.arctan2(y, x)
    t0, t1, t2, t3 = (atom_types[mapping[j]] for j in range(4))
    th = thetas[:, t0, t1, t2, t3]; kk = ks[:, t0, t1, t2, t3]
    degs = np.arange(1, 4)[:, None]
    V = np.sum(kk * (1.0 - np.cos(degs * theta[None, :] - th)), axis=0)
    return np.bincount(mapping_batch, weights=V.astype(np.float64),
                       minlength=256).astype(np.float32)


def kernel(pos, atom_types, mapping, mapping_batch, thetas, ks):
    from concourse.bass_utils import run_bass_kernel_spmd
    pos = np.asarray(pos, dtype=np.float32)
    atom_types = np.asarray(atom_types)
    mapping = np.asarray(mapping)
    mapping_batch = np.asarray(mapping_batch)
    thetas = np.asarray(thetas, dtype=np.float32)
    ks = np.asarray(ks, dtype=np.float32)

    base = np.asarray(mapping[0]).astype(np.int64)
    if not all(np.array_equal(np.asarray(mapping[j]), base + j)
               for j in range(1, 4)):
        print("kernel.py: non-consecutive mapping; numpy fallback",
              file=sys.stderr)
        return _kernel_numpy_fallback(pos, atom_types, mapping, mapping_batch,
                                      thetas, ks)

    inputs = dict(pos=pos, atom_types=atom_types, mapping=mapping,
                  mapping_batch=mapping_batch, thetas=thetas, ks=ks)
    plan, tables = prepare(inputs, F=1024, dtype="bf16")
    nc = build_program(plan["NTILES"], plan["F"], repeat=1,
                       dtype=plan["dtype"])
    in_maps = [{"tbl": tables[c]} for c in range(NCORES)]
    res = run_bass_kernel_spmd(nc, in_maps, list(range(NCORES)))
    outs = [res.results[c]["out"] for c in range(NCORES)]
    return finish(plan, outs).astype(np.float32)


# revision 19
# speedup vs baseline: 1.0418x; 1.0418x over previous
"""Trainium2 Bass kernel for nn_Dihedral (gnn_message_passing, 8 NeuronCores).

kernel(**inputs) -> [256] f32 per-batch dihedral energies.

Design v3 — gather-free streaming, engine-split compute. mapping columns are
consecutive-atom windows (b..b+3), so every per-dihedral quantity except the
batch label is a function of the window start. The host builds, per core, a
batch-sorted per-dihedral stream of 15 bf16 field planes
    [dr1 (3), dr2 (3), dr3 (3), A1..A3, B1'..B3']
(A = -k*cos(th0), B' = sign/2x-folded -k*sin(th0) — the same type-table fold
as v1; the angle-independent C = sum_k term is summed host-side) laid out
tile-contiguously in DRAM, so the device does pure SEQUENTIAL DMA (no
dma_gather, 30B/dihedral vs 256B in v1). Bins are padded to 32-element
blocks along partitions.

Device per core, per [128, F] tile: one dma_start pulls all 15 planes; the
columns are SPLIT between DVE (bf16 2x_1p) and gpsimd, each running the full
torsion chain independently on its slice (no cross-engine dependency
stalls); ACT does squares/sqrts/copies for both slices; PE reduces
32-partition blocks with a [128,4] indicator matmul. The torsion uses the
xy-polynomial form (cos/sin of d*phi expanded in x = n1.n2,
Y = (dr1.n2)|dr2|, u = 1/r — no trig tables). Host: bincount block sums
into 256 bins, sum the 8 per-core partials (the all-reduce of the
sum-sharded output).
"""

import os
import sys
import numpy as np

if "/opt/trn_rl_repo" not in sys.path:
    sys.path.insert(0, "/opt/trn_rl_repo")

import concourse.bass as bass
import concourse.bacc as bacc
import concourse.mybir as mybir
import concourse.tile as tile
from concourse.library_config import standard as std_lib
from concourse.tile_rust import add_dep_helper
import ml_dtypes

P = 128
NCORES = 8
QUANT = 32           # bin padding quantum == PE group size
NGRP = P // QUANT    # 4 partial sums per column
NPLANES = 15
NB = 256


# --------------------------------------------------------------------------
# host-side prep
# --------------------------------------------------------------------------

def build_window_fields(pos, atom_types, thetas, ks):
    """([15, NW] f32 per-window field planes, [NW] f32 C values)."""
    NW = pos.shape[0] - 3
    t3 = thetas.reshape(3, -1).astype(np.float64)
    k3 = ks.reshape(3, -1).astype(np.float64)
    A = (-(k3 * np.cos(t3))).astype(np.float32)      # [3, 390625]
    B = (-(k3 * np.sin(t3))).astype(np.float32)
    C = k3.sum(axis=0).astype(np.float32)
    ty = np.asarray(atom_types).astype(np.int64)
    T4 = ((ty[:NW] * 25 + ty[1:NW + 1]) * 25 + ty[2:NW + 2]) * 25 + ty[3:NW + 3]
    f = np.empty((NPLANES, NW), dtype=np.float32)
    f[0:3] = (pos[1:NW + 1] - pos[0:NW]).T
    f[3:6] = (pos[2:NW + 2] - pos[1:NW + 1]).T
    f[6:9] = (pos[3:NW + 3] - pos[2:NW + 2]).T
    f[9] = A[0, T4]; f[10] = A[1, T4]; f[11] = A[2, T4]
    f[12] = -B[0, T4]; f[13] = -2.0 * B[1, T4]; f[14] = -B[2, T4]
    return f, C[T4]


def plan_streams(base, batch, n_win, F):
    """Per-core batch-sorted window-index streams, bins padded to QUANT,
    common NTILES across cores."""
    SUBW = (n_win + NCORES - 1) // NCORES
    core_of = base // SUBW

    idx_streams = []
    lab_streams = []
    for c in range(NCORES):
        sel = np.nonzero(core_of == c)[0]        # batch-sorted already
        lab = batch[sel]
        cnt = np.bincount(lab, minlength=NB)
        pcnt = ((cnt + QUANT - 1) // QUANT) * QUANT
        total = int(pcnt.sum())
        idx_out = np.full(total, -1, dtype=np.int64)
        ends = np.cumsum(pcnt)
        starts = ends - pcnt
        within = np.arange(len(sel)) - np.repeat(np.cumsum(cnt) - cnt, cnt)
        idx_out[starts[lab] + within] = base[sel]
        blk_lab = np.repeat(np.arange(NB, dtype=np.int32), pcnt // QUANT)
        idx_streams.append(idx_out)
        lab_streams.append(blk_lab)

    max_cols = max((len(s) + P - 1) // P for s in idx_streams)
    NTILES = (max_cols + F - 1) // F
    NCOLS = NTILES * F

    streams = np.full((NCORES, NCOLS * P), -1, dtype=np.int64)
    blk_labels = np.full((NCORES, NCOLS * NGRP), -1, dtype=np.int32)
    for c in range(NCORES):
        streams[c, :len(idx_streams[c])] = idx_streams[c]
        blk_labels[c, :len(lab_streams[c])] = lab_streams[c]
    return streams, blk_labels, NTILES


def build_core_tables(fields, streams, NTILES, F, np_dtype):
    """[NCORES, NTILES*P, NPLANES*F] plane tables (tile-contiguous DRAM)."""
    tables = np.empty((NCORES, NTILES * P, NPLANES * F), dtype=np_dtype)
    for c in range(NCORES):
        w = streams[c]
        dummy = w < 0
        vals = fields[:, np.where(dummy, 0, w)]          # [15, NCOLS*P]
        if dummy.any():
            vals[9:15][:, dummy] = 0.0                   # V = 0 for padding
        # stream s = t*(P*F) + col*P + p  ->  dram[t*P + p, k*F + col]
        v4 = vals.reshape(NPLANES, NTILES, F, P)
        tables[c] = (v4.transpose(1, 3, 0, 2)
                     .reshape(NTILES * P, NPLANES * F).astype(np_dtype))
    return tables


# --------------------------------------------------------------------------
# device program
# --------------------------------------------------------------------------

def build_program(NTILES, F, repeat=1, dtype="bf16", split=None,
                  gat_bufs=None, tmp_bufs=None):
    if split is None:
        split = float(os.environ.get("DK_SPLIT", "1.0"))
    if gat_bufs is None:
        gat_bufs = int(os.environ.get("DK_GATB", "2"))
    if tmp_bufs is None:
        tmp_bufs = int(os.environ.get("DK_TMPB", "1"))
    f32 = mybir.dt.float32
    dt = mybir.dt.bfloat16 if dtype == "bf16" else f32
    Alu = mybir.AluOpType
    Act = mybir.ActivationFunctionType
    NCOLS = NTILES * F
    S = (int(F * split) // 4) * 4            # DVE columns; rest go to Pool

    nc = bacc.Bacc("TRN2", target_bir_lowering=False, debug=False)
    tbl = nc.dram_tensor("tbl", [NTILES * P, NPLANES * F], dt,
                         kind="ExternalInput").ap()
    out = nc.dram_tensor("out", [NGRP, NCOLS], f32, kind="ExternalOutput").ap()

    with tile.TileContext(nc) as tc:
        with (
            tc.tile_pool(name="gat", bufs=gat_bufs) as gat_pool,
            tc.tile_pool(name="tmp", bufs=tmp_bufs) as tmp_pool,
            tc.tile_pool(name="cst", bufs=1) as cst_pool,
            tc.tile_pool(name="ps", bufs=2, space="PSUM") as ps_pool,
        ):
            lib_inst = nc.gpsimd.load_library(std_lib)

            grp = cst_pool.tile([P, NGRP], dt)
            nc.gpsimd.memset(grp[:], 0.0)
            for g in range(NGRP):
                nc.gpsimd.memset(grp[g * QUANT:(g + 1) * QUANT, g:g + 1], 1.0)

            bs = cst_pool.tile([NGRP, NCOLS], f32)

            first_pool_op = [None]

            def do_side(g, side, lo, n):
                """Full torsion chain for columns [lo, lo+n) of tile g;
                tensor ops on DVE, squares/sqrt/affine on ACT."""
                def fld(k):
                    return g[:, k * F + lo:k * F + lo + n]

                def T(tag, d=dt):
                    return tmp_pool.tile([P, n], d, tag=f"{tag}_{side}",
                                         name=f"{tag}_{side}")

                def tt(o, i0, i1, op):
                    nc.vector.tensor_tensor(out=o, in0=i0, in1=i1, op=op)

                ax, ay, az = fld(0), fld(1), fld(2)
                bx, by, bz = fld(3), fld(4), fld(5)
                cx, cy, cz = fld(6), fld(7), fld(8)

                cs = T("cs")
                def cross1(o, m1, m2, m3, m4):
                    tt(o[:], m1, m2, Alu.mult)
                    tt(cs[:], m3, m4, Alu.mult)
                    tt(o[:], o[:], cs[:], Alu.subtract)

                n1x = T("n1x"); cross1(n1x, ay, bz, az, by)
                n1y = T("n1y"); cross1(n1y, az, bx, ax, bz)
                n1z = T("n1z"); cross1(n1z, ax, by, ay, bx)
                n2x = T("n2x"); cross1(n2x, by, cz, bz, cy)
                n2y = T("n2y"); cross1(n2y, bz, cx, bx, cz)
                n2z = T("n2z"); cross1(n2z, bx, cy, by, cx)

                def dot3(o, u0, u1, u2_, v0, v1, v2):
                    st = T("dt")
                    tt(o[:], u0, v0, Alu.mult)
                    tt(st[:], u1, v1, Alu.mult)
                    tt(o[:], o[:], st[:], Alu.add)
                    tt(st[:], u2_, v2, Alu.mult)
                    tt(o[:], o[:], st[:], Alu.add)

                x = T("x"); dot3(x, n1x[:], n1y[:], n1z[:], n2x[:], n2y[:], n2z[:])
                D = T("D"); dot3(D, ax, ay, az, n2x[:], n2y[:], n2z[:])

                w = T("w"); wt = T("wt")
                nc.scalar.square(w[:], bx)
                nc.scalar.square(wt[:], by)
                tt(w[:], w[:], wt[:], Alu.add)
                nc.scalar.square(wt[:], bz)
                tt(w[:], w[:], wt[:], Alu.add)
                L = T("L"); nc.scalar.sqrt(L[:], w[:])
                Y = T("Y"); tt(Y[:], D[:], L[:], Alu.mult)

                xx = T("xx"); nc.scalar.square(xx[:], x[:])
                YY = T("YY"); nc.scalar.square(YY[:], Y[:])
                q = T("q"); tt(q[:], xx[:], YY[:], Alu.add)
                r = T("r", f32); nc.scalar.sqrt(r[:], q[:])
                uf = T("uf", f32)
                nc.vector.reciprocal_approx_fast(uf[:], r[:])
                u = T("u"); nc.scalar.copy(u[:], uf[:])

                # normalized X = cos(phi), Yh = -sin(phi); X^2 + Yh^2 = 1
                Act_ = mybir.ActivationFunctionType
                X = T("X"); tt(X[:], x[:], u[:], Alu.mult)
                Yh = T("Yh"); tt(Yh[:], Y[:], u[:], Alu.mult)
                YY1 = T("YY1"); nc.scalar.square(YY1[:], Yh[:])
                c2 = T("c2")
                nc.scalar.activation(c2[:], YY1[:], Act_.Copy, bias=1.0,
                                     scale=-2.0)
                c3a = T("c3a")
                nc.scalar.activation(c3a[:], YY1[:], Act_.Copy, bias=1.0,
                                     scale=-4.0)
                s3a = T("s3a")
                nc.scalar.activation(s3a[:], YY1[:], Act_.Copy, bias=3.0,
                                     scale=-4.0)
                s2 = T("s2"); tt(s2[:], X[:], Yh[:], Alu.mult)
                c3 = T("c3"); tt(c3[:], c3a[:], X[:], Alu.mult)
                s3 = T("s3"); tt(s3[:], s3a[:], Yh[:], Alu.mult)

                # six Fourier terms; their sum is absorbed into the PSUM
                # accumulation of the block-sum matmuls
                npool = int(os.environ.get("DK_NPOOL", "0"))
                terms = []
                for i, (coef, trig) in enumerate([
                        (9, X), (12, Yh), (10, c2), (13, s2),
                        (11, c3), (14, s3)]):
                    tm = tmp_pool.tile([P, n], dt, tag=f"tm{i}_{side}",
                                       name=f"tm{i}_{side}")
                    if i < npool:
                        pi = nc.gpsimd.tensor_tensor(
                            out=tm[:], in0=fld(coef), in1=trig[:], op=Alu.mult)
                        if first_pool_op[0] is None:
                            first_pool_op[0] = pi
                            add_dep_helper(lib_inst.ins, pi.ins, sync=False,
                                           reason="library before pool ops")
                    else:
                        tt(tm[:], fld(coef), trig[:], Alu.mult)
                    terms.append(tm)
                return terms

            def do_tile(t):
                g = gat_pool.tile([P, NPLANES * F], dt, tag="g")
                rows = tbl[t * P:(t + 1) * P, :]
                for a, b in [(0, 6 * F), (6 * F, 9 * F), (9 * F, NPLANES * F)]:
                    nc.sync.dma_start(out=g[:, a:b], in_=rows[:, a:b])
                terms = do_side(g[:], "d", 0, F)
                pt = ps_pool.tile([NGRP, F], f32, tag="ps", name="ps")
                for c0 in range(0, F, 512):
                    c1 = min(c0 + 512, F)
                    for i, tm in enumerate(terms):
                        nc.tensor.matmul(out=pt[:, c0:c1], lhsT=grp[:],
                                         rhs=tm[:, c0:c1],
                                         start=(i == 0), stop=(i == 5))
                nc.scalar.activation(bs[:, t * F:(t + 1) * F], pt[:], Act.Copy)

            def body():
                for t in range(NTILES):
                    do_tile(t)

            if repeat > 1:
                with tc.For_i(0, repeat, 1):
                    body()
            else:
                body()

            nc.sync.dma_start(out=out[:], in_=bs[:])
    nc.compile()
    return nc


# --------------------------------------------------------------------------
# end to end
# --------------------------------------------------------------------------

def prepare(inputs, F=1024, dtype="bf16"):
    pos = np.asarray(inputs["pos"], dtype=np.float32)
    ty = np.asarray(inputs["atom_types"])
    mapping = np.asarray(inputs["mapping"])
    batch = np.asarray(inputs["mapping_batch"]).astype(np.int64)
    base = np.asarray(mapping[0]).astype(np.int64)
    assert all(np.array_equal(np.asarray(mapping[j]), base + j)
               for j in range(1, 4)), "mapping not consecutive; fast path invalid"
    n_win = pos.shape[0] - 3
    fields, Cw = build_window_fields(pos, ty, np.asarray(inputs["thetas"]),
                                     np.asarray(inputs["ks"]))
    streams, blk_labels, NTILES = plan_streams(base, batch, n_win, F)
    np_dtype = ml_dtypes.bfloat16 if dtype == "bf16" else np.float32
    tables = build_core_tables(fields, streams, NTILES, F, np_dtype)
    # angle-independent sum_k term, accumulated host-side
    energy_C = np.bincount(batch, weights=Cw[base].astype(np.float64),
                           minlength=NB)
    plan = dict(NTILES=NTILES, F=F, blk_labels=blk_labels, dtype=dtype,
                energy_C=energy_C)
    return plan, tables


def finish(plan, outs, n_batch=NB):
    """outs: list per core of [NGRP, NCOLS] block sums -> [256] energy."""
    energy = plan["energy_C"].copy()
    for c in range(NCORES):
        bsums = np.asarray(outs[c])          # [NGRP, NCOLS]
        lab = plan["blk_labels"][c]          # [NCOLS*NGRP], -1 = padding
        vals = bsums.T.ravel()               # block (col, grp) order
        m = lab >= 0
        energy += np.bincount(lab[m], weights=vals[m].astype(np.float64),
                              minlength=n_batch)
    return energy.astype(np.float32)


def _kernel_numpy_fallback(pos, atom_types, mapping, mapping_batch, thetas, ks):
    # Correctness safety net for non-consecutive mappings (never expected).
    p0, p1 = pos[mapping[0]], pos[mapping[1]]
    p2, p3 = pos[mapping[2]], pos[mapping[3]]
    dr1, dr2, dr3 = p1 - p0, p2 - p1, p3 - p2
    n1 = np.cross(dr1, dr2); n2 = np.cross(dr2, dr3)
    m1 = np.cross(n1, dr2 / np.linalg.norm(dr2, axis=-1, keepdims=True))
    x = np.sum(n1 * n2, -1); y = np.sum(m1 * n2, -1)
    theta = np.arctan2(y, x)
    t0, t1, t2, t3 = (atom_types[mapping[j]] for j in range(4))
    th = thetas[:, t0, t1, t2, t3]; kk = ks[:, t0, t1, t2, t3]
    degs = np.arange(1, 4)[:, None]
    V = np.sum(kk * (1.0 - np.cos(degs * theta[None, :] - th)), axis=0)
    return np.bincount(mapping_batch, weights=V.astype(np.float64),
                       minlength=256).astype(np.float32)


def kernel(pos, atom_types, mapping, mapping_batch, thetas, ks):
    from concourse.bass_utils import run_bass_kernel_spmd
    pos = np.asarray(pos, dtype=np.float32)
    atom_types = np.asarray(atom_types)
    mapping = np.asarray(mapping)
    mapping_batch = np.asarray(mapping_batch)
    thetas = np.asarray(thetas, dtype=np.float32)
    ks = np.asarray(ks, dtype=np.float32)

    base = np.asarray(mapping[0]).astype(np.int64)
    if not all(np.array_equal(np.asarray(mapping[j]), base + j)
               for j in range(1, 4)):
        print("kernel.py: non-consecutive mapping; numpy fallback",
              file=sys.stderr)
        return _kernel_numpy_fallback(pos, atom_types, mapping, mapping_batch,
                                      thetas, ks)

    inputs = dict(pos=pos, atom_types=atom_types, mapping=mapping,
                  mapping_batch=mapping_batch, thetas=thetas, ks=ks)
    plan, tables = prepare(inputs, F=1024, dtype="bf16")
    nc = build_program(plan["NTILES"], plan["F"], repeat=1,
                       dtype=plan["dtype"])
    in_maps = [{"tbl": tables[c]} for c in range(NCORES)]
    res = run_bass_kernel_spmd(nc, in_maps, list(range(NCORES)))
    outs = [res.results[c]["out"] for c in range(NCORES)]
    return finish(plan, outs).astype(np.float32)
